# revision 3
# baseline (speedup 1.0000x reference)
"""Trainium2 Bass kernel v3 for DatasetIndexedTopK (streaming top-k retrieval).

Single-DVE-scan screen via byte-interleaved (quant, position) packing:

  - host supplies bf16 qT/candT; PE runs full-rate bf16 matmuls into PSUM.
  - ACT drains PSUM with a fused quantize q8 = round(s + 128) (u16 out),
    writing the HIGH halfword of each u32 lane of a packed tile (stride-2
    u16 access pattern).  The LOW halfwords were pre-filled once with
    revpos = 4095 - pos_in_block; they survive because ACT only ever
    touches the odd halfwords of the statically-allocated packed tiles.
  - DVE max8 over the u32 view compares (quant, revpos) lexicographically:
    top-8 per 4096-block with positions riding in the low bits.  packed
    values stay < 2^24 (quant <= 255), so the fp32-internal comparator is
    exact.  The baseline's max_index pass and any arithmetic pack op are
    eliminated: DVE does exactly one 1-cyc/elem scan (~562 us/core) and
    nothing else.
  - per-core summaries [512, 256] u32 go to the host, which merges cores,
    takes top-M=256 per query by packed value, rescores exactly in fp32,
    and emits exact top-k (ties -> lower id, matching the reference).

Screen safety (quant quantum 1.0): a true top-100 member is lost only if
>=8 blockmates quantize >= it, i.e. ~Poisson(0.6) tail >=8 ~ 1e-8 per
member-event; and the host rescore cut M=256 covers count(score >
t100-1) ~ 140 +- 12 with 9 sigma of margin.
"""

import numpy as np

P = 128
D = 128
Q = 512
NCORES = 8
NCAND_TOTAL = 256 * 4096
NCAND = NCAND_TOTAL // NCORES      # 131072 per core
CTILE = 8192                       # candidate columns per DMA tile (bf16)
PTILE = 2048                       # columns per PSUM tile (4 banks)
BLK = 4096                         # candidates per packed max8 block
NBLK = NCAND // BLK                # 32 blocks per chunk
NCHUNK = Q // P                    # 4 query chunks
S_W = NBLK * 8                     # summary width per chunk (256)
NPK = 4                            # packed-tile rotation depth
M_RESCORE = 256                    # host rescore depth per query

_CACHE = {}


def _build_bass_v3(repeat=1):
    import concourse.bacc as bacc
    import concourse.mybir as mybir
    from concourse.tile import TileContext
    from contextlib import ExitStack

    f32 = mybir.dt.float32
    u16 = mybir.dt.uint16
    u32 = mybir.dt.uint32
    bf16 = mybir.dt.bfloat16
    AF = mybir.ActivationFunctionType

    nc = bacc.Bacc()
    qT = nc.declare_dram_parameter("qT", [D, Q], bf16, isOutput=False)
    candT = nc.declare_dram_parameter("candT", [D, NCAND], bf16, isOutput=False)
    iot = nc.declare_dram_parameter("iot", [P, BLK], u16, isOutput=False)
    s_out = nc.declare_dram_parameter("s_out", [Q, S_W], u32, isOutput=True)

    with ExitStack() as ctx:
        tc = ctx.enter_context(TileContext(nc))
        qpool = ctx.enter_context(tc.tile_pool(name="q", bufs=1))
        cpool = ctx.enter_context(tc.tile_pool(name="cand", bufs=3))
        pspool = ctx.enter_context(tc.tile_pool(name="ps", bufs=2, space="PSUM"))
        acc = ctx.enter_context(tc.tile_pool(name="acc", bufs=1))

        qsb = qpool.tile([D, Q], bf16, tag="qsb")
        nc.sync.dma_start(qsb[:], qT[:])
        iota = qpool.tile([P, BLK], u16, tag="iota")
        nc.sync.dma_start(iota[:], iot[:])

        # Static packed tiles: low halfwords = revpos iota (written once),
        # high halfwords = quant (rewritten each use by ACT).
        pks = [acc.tile([P, BLK], u32, tag=f"pk{i}", name=f"pk{i}")
               for i in range(NPK)]
        for pk in pks:
            nc.vector.tensor_copy(pk[:].bitcast(u16)[:, 0:2 * BLK:2], iota[:])

        S_all = acc.tile([P, NCHUNK * S_W], u32, tag="S")

        NT = NCAND // CTILE
        it = 0
        for t in range(repeat * NT):
            t = t % NT
            ct = cpool.tile([D, CTILE], bf16, tag="cand")
            nc.sync.dma_start(ct[:], candT[:, t * CTILE:(t + 1) * CTILE])
            for qc in range(NCHUNK):
                for b2 in range(CTILE // BLK):
                    pk = pks[it % NPK]
                    it += 1
                    pku16 = pk[:].bitcast(u16)
                    for p in range(BLK // PTILE):
                        ps = pspool.tile([P, PTILE], f32, tag="ps")
                        for j in range(PTILE // 512):
                            col = b2 * BLK + p * PTILE + j * 512
                            nc.tensor.matmul(
                                ps[:, j * 512:(j + 1) * 512],
                                lhsT=qsb[:, qc * P:(qc + 1) * P],
                                rhs=ct[:, col:col + 512],
                                start=True,
                                stop=True,
                            )
                        nc.scalar.activation(
                            pku16[:, 2 * p * PTILE + 1:2 * (p + 1) * PTILE:2],
                            ps[:], AF.Copy, bias=128.0, scale=1.0)
                    b = t * (CTILE // BLK) + b2       # global block 0..31
                    so = qc * S_W + b * 8
                    nc.vector.max(out=S_all[:, so:so + 8], in_=pk[:])

        for qc in range(NCHUNK):
            nc.sync.dma_start(
                s_out[qc * P:(qc + 1) * P, :],
                S_all[:, qc * S_W:(qc + 1) * S_W])
    nc.compile()
    return nc


def _get_bass():
    if "nc" not in _CACHE:
        _CACHE["nc"] = _build_bass_v3()
    return _CACHE["nc"]


def kernel(query_embeddings, candidate_embeddings, candidate_indices, k):
    from concourse.bass_utils import run_bass_kernel_spmd
    import concourse.mybir as mybir

    q = np.asarray(query_embeddings, dtype=np.float32)          # [512, 128]
    c = np.asarray(candidate_embeddings, dtype=np.float32).reshape(NCAND_TOTAL, D)
    ids_flat = np.asarray(candidate_indices).reshape(-1)
    k = int(k)
    assert k <= M_RESCORE, f"k={k} exceeds host rescore depth {M_RESCORE}"

    bf16 = mybir.dt.np(mybir.dt.bfloat16)
    qT = np.ascontiguousarray(q.T).astype(bf16)                  # [128, 512]
    iota = np.tile((BLK - 1 - np.arange(BLK)).astype(np.uint16), (P, 1))
    in_maps = []
    for core in range(NCORES):
        shard = c[core * NCAND:(core + 1) * NCAND]               # [131072, 128]
        in_maps.append({
            "qT": qT,
            "candT": np.ascontiguousarray(shard.T).astype(bf16),
            "iot": iota,
        })

    nc = _get_bass()
    res = run_bass_kernel_spmd(nc, in_maps, core_ids=list(range(NCORES))).results

    # ---- host: decode summaries, merge cores, exact rescore ----
    S = np.stack([res[m]["s_out"] for m in range(NCORES)]).astype(np.int64)
    blk = (np.arange(S_W) // 8)[None, None, :]
    pos = (BLK - 1) - (S & 0xFFFF)
    gpos = np.arange(NCORES)[:, None, None] * NCAND + blk * BLK + pos
    Sq = np.concatenate(list(S), axis=1)                         # [512, 2048]
    Gq = np.concatenate(list(gpos), axis=1)

    M = min(M_RESCORE, Sq.shape[1])
    part = np.argpartition(-Sq, M - 1, axis=1)[:, :M]
    cand_idx = np.take_along_axis(Gq, part, axis=1)              # [512, M]

    vecs = c[cand_idx]                                           # [512, M, 128]
    scores = np.einsum("qd,qmd->qm", q, vecs, dtype=np.float32).astype(np.float32)

    out_scores = np.empty((Q, k), dtype=np.float32)
    out_pos = np.empty((Q, k), dtype=np.int64)
    for qi in range(Q):
        order = np.lexsort((cand_idx[qi], -scores[qi]))[:k]
        out_scores[qi] = scores[qi, order]
        out_pos[qi] = cand_idx[qi, order]

    out_ids = ids_flat[out_pos].astype(ids_flat.dtype)
    return out_scores, out_ids
